# revision 17
# baseline (speedup 1.0000x reference)
"""Multi-head attention (N=2, T=2048, D=1024, H=16, dk=dv=64) on 8 TRN2 cores.

Sharding: tensor-parallel over heads. Core p computes heads {2p, 2p+1}
(a 128-wide slice of the QKV projections and of WO's rows), producing a
partial output [2, 2048, 1024]; the host sums the 8 partials and adds bO
(row-parallel linear => sum-reduce unshard).

Device algorithm (per core, per batch n):
  1. qT = (WQp/8).T @ Q.T   [128, 2048]   (scale 1/sqrt(dk) folded into WQp)
     kT = WKp.T @ K.T       [128, 2048]
     v  = V @ WVp           [128part(l-tile), 16, 2, dk+1] with a ones
          column appended per head (softmax denominator for free)
  2. scores in "KQ" orientation: S^T[l, q] = kT.T(l-tile) @ qT(q-chunk);
     the two heads run CONCURRENTLY on PE row groups 0-63 / 64-127.
  3. E = exp(S^T) on ScalarE (its only job), PSUM -> SBUF bf16.
  4. attnT_aug[dv+1, q] += v_aug.T(l-tile) @ E accumulated over l-tiles in
     PSUM; row dv holds sum(exp) = softmax denominator.
  5. normalize: att = attnT * bcast(1/den) (DVE; 1/den broadcast across
     partitions via a K=1 PE matmul)
  6. O^T-partial: out[q-tile, :] = att[:, q-tile].T @ WOp, written fp16.

Schedule: a warm-up burst of dummy matmuls un-throttles the PE HAM clock
gate (4/8 -> 8/8) during the initial DMA wait; after that every piece of
projection / normalize / out-projection work is dripped into the
exp-paced attention l-loops so the PE stream never idles (no HAM
re-throttle) and ScalarE runs an uninterrupted exp stream.  All
PSUM->SBUF casts are on DVE, all DMA issue on the SP/GpSimd queues.
"""

import math
import numpy as np
from contextlib import ExitStack
from collections import deque

import concourse.bass as bass
import concourse.tile as tile
from concourse import bacc, mybir
from concourse.bass_utils import run_bass_kernel_spmd

N_CORES = 8
NB, T, D = 2, 2048, 1024
HEADS, DK = 16, 64
HP = 2 * DK          # per-core head-pair width = 128
QC = 512             # query-chunk (matmul moving free dim)
NQC = T // QC        # 4
LTS = 128            # key/l tile (PE partition dim)
NLT = T // LTS       # 16
CK = 128             # contraction chunk for projections
NCK = D // CK        # 8
VW = DK + 1          # v columns per head incl. ones column

F32 = mybir.dt.float32
BF16 = mybir.dt.bfloat16
FP16 = mybir.dt.float16
EXP = mybir.ActivationFunctionType.Exp


def build_program(mm_dt=BF16, out_dt=FP16):
    """Build + compile the SPMD program (identical on all 8 cores)."""
    nc = bacc.Bacc("TRN2", target_bir_lowering=False, debug=False,
                   num_devices=N_CORES)
    # [NB, NQC, CK, NCK*QC]: per (n, chunk) a contiguous [128, 8KB] block
    # so each input chunk is ONE dma_start with 2KB+ descriptors.
    QT = nc.dram_tensor("QT", [NB, NQC, CK, NCK, QC], mm_dt,
                        kind="ExternalInput").ap()
    KT = nc.dram_tensor("KT", [NB, NQC, CK, NCK, QC], mm_dt,
                        kind="ExternalInput").ap()
    VT = nc.dram_tensor("VT", [NB, NQC, CK, NCK, QC], mm_dt,
                        kind="ExternalInput").ap()
    WQp = nc.dram_tensor("WQp", [CK, NCK, HP], mm_dt,
                         kind="ExternalInput").ap()
    WKp = nc.dram_tensor("WKp", [CK, NCK, HP], mm_dt,
                         kind="ExternalInput").ap()
    WVp = nc.dram_tensor("WVp", [CK, NCK, HP], mm_dt,
                         kind="ExternalInput").ap()
    WOp = nc.dram_tensor("WOp", [HP, D], mm_dt, kind="ExternalInput").ap()
    O = nc.dram_tensor("O", [NB, T, D], out_dt, kind="ExternalOutput").ap()

    with tile.TileContext(nc) as tc, ExitStack() as ctx:
        wpool = ctx.enter_context(tc.tile_pool(name="w", bufs=1))
        seq = ctx.enter_context(tc.tile_pool(name="seq", bufs=2))
        inp = ctx.enter_context(tc.tile_pool(name="inp", bufs=11))
        epool = ctx.enter_context(tc.tile_pool(name="e", bufs=8))
        apool = ctx.enter_context(tc.tile_pool(name="att", bufs=3))
        opool = ctx.enter_context(tc.tile_pool(name="o", bufs=8))
        ppool = ctx.enter_context(tc.tile_pool(name="pp", bufs=2, space="PSUM"))
        spool = ctx.enter_context(tc.tile_pool(name="ps", bufs=2, space="PSUM"))
        atpool = ctx.enter_context(tc.tile_pool(name="pa", bufs=1, space="PSUM"))

        # --- static SBUF: weights + constants ---
        wq_s = wpool.tile([CK, NCK, HP], mm_dt)
        wk_s = wpool.tile([CK, NCK, HP], mm_dt)
        wv_s = wpool.tile([CK, NCK, HP], mm_dt)
        wo_s = wpool.tile([HP, D], mm_dt)

        ones_col = wpool.tile([1, DK], F32, name="ones_col")
        nc.vector.memset(ones_col, 1.0)
        ones_col_r = wpool.tile([1, DK], mm_dt, name="ones_col_r")
        nc.vector.tensor_copy(ones_col_r, ones_col)
        # warm-up operand (zeros are fine for dummy matmuls)
        wsrc = wpool.tile([CK, QC], mm_dt, name="wsrc")
        nc.vector.memset(wsrc, 0.125)

        # --- HAM warm-up: dummy matmuls that run during the initial DMA
        # wait. ~8 N=512 matmuls at the cold 1.2GHz clock cover the 3.4us
        # busy window that flips the PE clock gate to 8/8; a few more keep
        # it busy until the first real projection's inputs land. ---
        for wi in range(10):
            pw = ppool.tile([HP, QC], F32, tag="pp", name="warm")
            nc.tensor.matmul(pw, lhsT=wsrc[:, 0:HP], rhs=wsrc,
                             start=True, stop=True)

        # --- staged input chunks (one tile + one DMA per (src, n, c)) ---
        stage = {}

        def dma_in(src, n, c, key, eng=None):
            # 8 per-ck tiles + sub-DMAs per chunk: each issue-stream only
            # sustains ~45GB/s so landing latency scales with issue count,
            # and per-ck tiles let each projection matmul start as soon as
            # its own 128KB block lands (per-tile dependency tracking).
            def f():
                q = eng or nc.sync
                for ck in range(NCK):
                    cin = inp.tile([CK, QC], mm_dt, tag="cin", bufs=88,
                                   name="cin")
                    q.dma_start(out=cin, in_=src[n, c, :, ck, :])
                    stage[(key, ck)] = cin
            return f

        def qk_mm(w_s, dst, qc, key):
            """Fused projection: 8 accumulating matmuls + one DVE cast."""
            def f():
                ps = ppool.tile([HP, QC], F32, tag="pp", name="ps_proj")
                for ck in range(NCK):
                    nc.tensor.matmul(ps, lhsT=w_s[:, ck, :],
                                     rhs=stage.pop((key, ck)),
                                     start=(ck == 0), stop=(ck == NCK - 1))
                nc.vector.tensor_copy(dst[:, qc * QC:(qc + 1) * QC], ps)
            return [f]

        def v_mm(v_sb, n, c, key):
            """v natural-layout projection for token-chunk c (4 l-tiles);
            one [128, 2, 64] DVE copy per l-tile (ones cols untouched)."""
            cins = {}
            def grab():
                for ck in range(NCK):
                    cins[ck] = stage.pop((key, ck))
            def t(j):
                def f():
                    lt = c * (QC // LTS) + j
                    pv = ppool.tile([LTS, 2, DK], F32, tag="pp", name="pv")
                    for ck in range(NCK):
                        nc.tensor.matmul(
                            pv, lhsT=cins[ck][:, j * LTS:(j + 1) * LTS],
                            rhs=wv_s[:, ck, :],
                            start=(ck == 0), stop=(ck == NCK - 1))
                    nc.vector.tensor_copy(v_sb[:, lt, :, 0:DK], pv)
                return f
            return [grab] + [t(j) for j in range(QC // LTS)]

        def scores_pair(qT_sb, kT_sb, qc, lt):
            """S^T for both heads of (q-chunk, l-tile) into one 2-bank PSUM
            tile; heads run concurrently on PE row groups; single wide exp."""
            ss = spool.tile([LTS, 2 * QC], F32, tag="ss", name="ss")
            for h in range(2):
                nc.tensor.matmul(
                    ss[:, h * QC:(h + 1) * QC],
                    lhsT=kT_sb[DK * h:DK * (h + 1), lt * LTS:(lt + 1) * LTS],
                    rhs=qT_sb[DK * h:DK * (h + 1), qc * QC:(qc + 1) * QC],
                    start=True, stop=True)
            e = epool.tile([LTS, 2 * QC], mm_dt, tag="e", name="e")
            nc.scalar.activation(e, ss, EXP)
            return e

        def av_pair(v_sb, ps_att, e, lt, start, stop):
            for h in range(2):
                nc.tensor.matmul(ps_att[h],
                                 lhsT=v_sb[:, lt, h, :],
                                 rhs=e[:, h * QC:(h + 1) * QC],
                                 start=start, stop=stop)

        def attention_chunk(qT_sb, kT_sb, v_sb, qc, work, slots=None):
            """Emit attention for one q-chunk; `slots` maps l-tile index ->
            thunks that MUST be emitted at that point (production
            deadlines); `work` thunks are consumed evenly across the
            l-tile loop."""
            slots = slots or {}
            ps_att = [atpool.tile([VW, QC], F32, tag=f"pa{h}",
                                  name=f"ps_att{h}") for h in range(2)]
            prev = None
            for lt in range(NLT):
                e = scores_pair(qT_sb, kT_sb, qc, lt)
                for t in slots.get(lt, ()):
                    t()
                if work:
                    take = -(-len(work) // (NLT - lt))
                    for _ in range(min(take, len(work))):
                        work.popleft()()
                if prev is not None:
                    av_pair(v_sb, ps_att, prev, lt - 1, start=(lt == 1),
                            stop=False)
                prev = e
            av_pair(v_sb, ps_att, prev, NLT - 1, start=(NLT == 1), stop=True)
            while work:
                work.popleft()()
            return ps_att

        def norm_thunks(ps_att):
            """Softmax normalization for a finished accumulator pair. The
            PSUM quick-release copies (tA) are emitted INLINE so the next
            chunk's first AV matmul (which reuses the single-buffered
            accumulator) never convoys behind later DVE work. Returns
            (att, [C, D], tDj): C builds the 1/den broadcast, D applies."""
            att_raw = apool.tile([HP, QC], F32, tag="att_raw", name="att_raw")
            att = apool.tile([HP, QC], mm_dt, tag="attT", name="att")
            state = {}

            def tA(h):
                def f():
                    nc.vector.tensor_copy(att_raw[DK * h:DK * (h + 1), :],
                                          ps_att[h][0:DK, :])
                    den_f = apool.tile([1, QC], F32, tag=f"den{h}",
                                       name="den_f")
                    nc.vector.tensor_copy(den_f, ps_att[h][DK:VW, :])
                    state["den%d" % h] = den_f
                return f

            def tC():
                for h in range(2):
                    den_rf = apool.tile([1, QC], F32, tag=f"denr{h}",
                                        name="den_rf")
                    nc.vector.reciprocal_approx_fast(den_rf,
                                                     state["den%d" % h])
                    den_rr = apool.tile([1, QC], mm_dt, tag=f"denrr{h}",
                                        name="den_rr")
                    nc.vector.tensor_copy(den_rr, den_rf)
                    bcp = ppool.tile([DK, QC], F32, tag="pp", name="bc_ps")
                    nc.tensor.matmul(bcp, lhsT=ones_col_r, rhs=den_rr,
                                     start=True, stop=True)
                    state["bc%d" % h] = bcp

            def tD():
                for h in range(2):
                    nc.vector.tensor_mul(att[DK * h:DK * (h + 1), :],
                                         att_raw[DK * h:DK * (h + 1), :],
                                         state["bc%d" % h])

            def tDj(j):
                sl = slice(j * LTS, (j + 1) * LTS)
                for h in range(2):
                    nc.vector.tensor_mul(att[DK * h:DK * (h + 1), sl],
                                         att_raw[DK * h:DK * (h + 1), sl],
                                         state["bc%d" % h][:, sl])

            tA(0)()
            tA(1)()
            return att, [tC, tD], tDj

        def out_proj_thunks(n, att, qc, queues=None):
            """out-projection chunk: 8 (MM + fp16-cast) thunks, DMA per
            q-tile issued from `queues` (round-robin)."""
            queues = queues or [nc.gpsimd]
            box = {}
            thunks = []
            for j in range(QC // LTS):
                for half in range(2):
                    def t(j=j, half=half):
                        qt = qc * (QC // LTS) + j
                        if half == 0:
                            box[j] = opool.tile([LTS, D], out_dt, tag="osb",
                                                name="o_sb")
                        o_sb = box[j]
                        po = ppool.tile([LTS, QC], F32, tag="pp", name="po")
                        nc.tensor.matmul(
                            po, lhsT=att[:, j * LTS:(j + 1) * LTS],
                            rhs=wo_s[:, half * QC:(half + 1) * QC],
                            start=True, stop=True)
                        nc.vector.tensor_copy(
                            o_sb[:, half * QC:(half + 1) * QC], po)
                        if half == 1:
                            queues[j % len(queues)].dma_start(
                                out=O[n, qt * LTS:(qt + 1) * LTS, :],
                                in_=box.pop(j))
                    thunks.append(t)
            return thunks

        def body():
            seqs = []
            for n in range(NB):
                qT_sb = seq.tile([HP, T], mm_dt, tag="qT", name="qT_sb")
                kT_sb = seq.tile([HP, T], mm_dt, tag="kT", name="kT_sb")
                v_sb = seq.tile([LTS, NLT, 2, VW], mm_dt, tag="v",
                                name="v_sb")
                nc.vector.memset(v_sb[:, :, :, DK], 1.0)
                seqs.append((qT_sb, kT_sb, v_sb))

            def kv_proj(n, c):
                """3 drip thunks: k-proj burst, then v j-tiles 0-1, 2-3."""
                k = qk_mm(wk_s, seqs[n][1], c, ("k", n, c))
                v = v_mm(seqs[n][2], n, c, ("v", n, c))
                return [k[0], lambda: [t() for t in v[0:3]],
                        lambda: [t() for t in v[3:5]]]

            def dmas(*specs):
                """One thunk per chunk DMA (8 issues each) on gpsimd."""
                return [dma_in(src, n, c, (pfx, n, c), eng=nc.gpsimd)
                        for (src, n, c, pfx) in specs]

            # --- startup: Q/K chunk-0 on the SP queue, V chunk-0 + chunk-1
            # K/V on the GpSimd queue (both queues pull in parallel);
            # everything later is issued from GpSimd inside the l-loops.
            # Warm-up matmuls above cover the DMA wait; the three chunk-0
            # projections run as soon as their input lands. ---
            nc.sync.dma_start(out=wq_s, in_=WQp)
            dma_in(QT, 0, 0, ("q", 0, 0))()
            nc.sync.dma_start(out=wk_s, in_=WKp)
            dma_in(KT, 0, 0, ("k", 0, 0))()
            nc.gpsimd.dma_start(out=wv_s, in_=WVp)
            dma_in(VT, 0, 0, ("v", 0, 0), eng=nc.gpsimd)()
            dma_in(KT, 0, 1, ("k", 0, 1), eng=nc.gpsimd)()
            dma_in(VT, 0, 1, ("v", 0, 1), eng=nc.gpsimd)()
            nc.gpsimd.dma_start(out=wo_s, in_=WOp)

            for t in qk_mm(wq_s, seqs[0][0], 0, ("q", 0, 0)):
                t()
            for t in qk_mm(wk_s, seqs[0][1], 0, ("k", 0, 0)):
                t()
            for t in v_mm(seqs[0][2], 0, 0, ("v", 0, 0)):
                t()

            # per-chunk deadline slots and drip work
            kv01 = kv_proj(0, 1)
            kv02 = kv_proj(0, 2)
            kv03 = kv_proj(0, 3)
            slots_ci0 = {
                0: dmas((KT, 0, 2, "k"), (VT, 0, 2, "v")),
                2: [kv01[0]],
                4: [kv01[1]],
                5: [kv01[2]],
                3: dmas((KT, 0, 3, "k"), (VT, 0, 3, "v"), (QT, 0, 1, "q")),
                6: [kv02[0]],
                7: [kv02[1]],
                8: [kv02[2]],
                9: dmas((KT, 1, 0, "k"), (VT, 1, 0, "v")),
                10: [kv03[0]],
                11: [kv03[1]],
                12: [kv03[2]],
                13: qk_mm(wq_s, seqs[0][0], 1, ("q", 0, 1)),
                14: dmas((QT, 0, 2, "q"), (KT, 1, 1, "k"), (VT, 1, 1, "v")),
            }
            # extra DMA issues dripped in later chunks; the SP queue is
            # idle mid-run so prefetch there, keeping gpsimd free for the
            # out-projection writebacks.
            def dmas_sp(*specs):
                return [dma_in(src, n, c, (pfx, n, c))
                        for (src, n, c, pfx) in specs]
            dma_extra = {
                1: dmas_sp((KT, 1, 2, "k"), (VT, 1, 2, "v"), (QT, 0, 3, "q")),
                2: dmas_sp((KT, 1, 3, "k"), (VT, 1, 3, "v"), (QT, 1, 0, "q")),
                3: dmas_sp((QT, 1, 1, "q")),
                4: dmas_sp((QT, 1, 2, "q")),
                5: dmas_sp((QT, 1, 3, "q")),
            }
            # future projection work dripped per chunk index ci=1..7
            proj_extra = {
                1: qk_mm(wq_s, seqs[0][0], 2, ("q", 0, 2)) + kv_proj(1, 0),
                2: qk_mm(wq_s, seqs[0][0], 3, ("q", 0, 3)) + kv_proj(1, 1),
                3: qk_mm(wq_s, seqs[1][0], 0, ("q", 1, 0)) + kv_proj(1, 2),
                4: qk_mm(wq_s, seqs[1][0], 1, ("q", 1, 1)),
                5: qk_mm(wq_s, seqs[1][0], 2, ("q", 1, 2)),
                6: qk_mm(wq_s, seqs[1][0], 3, ("q", 1, 3)),
            }
            # batch-1 chunk-3 K/V projection has a hard deadline (scores of
            # chunk (1,0) l-tiles 12-15): pin it in slots of ci=4.
            kv13 = kv_proj(1, 3)
            slots_ci4 = {0: [kv13[0]], 1: [kv13[1]], 2: [kv13[2]]}

            pend_norm = None
            pend_out = None
            for ci in range(NB * NQC):
                n, qc = divmod(ci, NQC)
                qT_sb, kT_sb, v_sb = seqs[n]
                work = deque()
                if pend_norm:
                    work.extend(pend_norm)          # bc + mul
                work.extend(dma_extra.get(ci, ()))
                if pend_out is not None:
                    work.extend(out_proj_thunks(pend_out[2], pend_out[0],
                                                pend_out[1]))
                work.extend(proj_extra.get(ci, ()))
                slots = slots_ci0 if ci == 0 else \
                    (slots_ci4 if ci == 4 else None)
                ps_att = attention_chunk(qT_sb, kT_sb, v_sb, qc, work, slots)
                att, pend_norm, pend_tdj = norm_thunks(ps_att)
                pend_out = (att, qc, n)
            # pipelined tail: per q-tile, normalize the slice then
            # immediately out-project it; final DMAs alternate queues.
            # Dummy matmuls keep the PE HAM clock gate at 8/8 through the
            # norm-chain waits so the out-projection runs at 2.4GHz.
            def warm(k):
                for _ in range(k):
                    pw = ppool.tile([HP, QC], F32, tag="pp", name="warm")
                    nc.tensor.matmul(pw, lhsT=wsrc[:, 0:HP], rhs=wsrc,
                                     start=True, stop=True)
            warm(4)                     # PE busy while DVE runs tA + recip
            pend_norm[0]()              # tC (bc matmuls)
            pend_norm[1]()              # tD
            warm(6)                     # PE busy while DVE runs tD
            op = out_proj_thunks(pend_out[2], pend_out[0], pend_out[1],
                                 queues=[nc.gpsimd, nc.sync])
            for j in range(QC // LTS):
                op[2 * j]()
                op[2 * j + 1]()
                warm(2)

        body()

    nc.compile()
    return nc


_CACHED = {}


def _get_program(key=("bf16",)):
    if key not in _CACHED:
        _CACHED[key] = build_program()
    return _CACHED[key]


def prep_inputs(Q, K, V, WQ, WK, WV, WO, bO):
    """Host-side shard prep: transposes + per-core weight slices."""
    import ml_dtypes
    wire = ml_dtypes.bfloat16
    Q = np.asarray(Q, dtype=np.float32)
    K = np.asarray(K, dtype=np.float32)
    V = np.asarray(V, dtype=np.float32)
    WQ = np.asarray(WQ, dtype=np.float32)
    WK = np.asarray(WK, dtype=np.float32)
    WV = np.asarray(WV, dtype=np.float32)
    WO = np.asarray(WO, dtype=np.float32)

    def blockT(X):
        # [N, T, D] -> X^T blocked [NB, NQC, CK, NCK, QC]; per (n, qc) the
        # [CK, NCK*QC] block is contiguous (one DMA, 8KB per partition)
        Xt = np.swapaxes(X, 1, 2).reshape(NB, NCK, CK, NQC, QC)
        return np.ascontiguousarray(
            Xt.transpose(0, 3, 2, 1, 4)).astype(wire)

    QT = blockT(Q)
    KT = blockT(K)
    VT = blockT(V)
    scale = 1.0 / math.sqrt(DK)

    def wblk(w):
        # [D, HP] -> [CK, NCK, HP] (d = k*CK + c -> [c, k, m]), contiguous
        return np.ascontiguousarray(
            w.reshape(NCK, CK, HP).transpose(1, 0, 2)).astype(wire)

    in_maps = []
    for p in range(N_CORES):
        sl = slice(HP * p, HP * (p + 1))
        in_maps.append({
            "QT": QT, "KT": KT, "VT": VT,
            "WQp": wblk(np.ascontiguousarray(WQ[:, sl]) * scale),
            "WKp": wblk(np.ascontiguousarray(WK[:, sl])),
            "WVp": wblk(np.ascontiguousarray(WV[:, sl])),
            "WOp": np.ascontiguousarray(WO[sl, :]).astype(wire),
        })
    return in_maps


def kernel(Q, K, V, WQ, WK, WV, WO, bO):
    nc = _get_program()
    in_maps = prep_inputs(Q, K, V, WQ, WK, WV, WO, bO)
    res = run_bass_kernel_spmd(nc, in_maps, list(range(N_CORES)))
    acc = np.zeros((NB, T, D), np.float32)
    for p in range(N_CORES):
        acc += res.results[p]["O"].astype(np.float32)
    return acc + np.asarray(bO, dtype=np.float32)


# revision 20
# speedup vs baseline: 1.1911x; 1.1911x over previous
"""Multi-head attention (N=2, T=2048, D=1024, H=16, dk=dv=64) on 8 TRN2 cores.

Sharding: tensor-parallel over heads. Core p computes heads {2p, 2p+1}
(a 128-wide slice of the QKV projections and of WO's rows), producing a
partial output [2, 2048, 1024]; the host sums the 8 partials and adds bO
(row-parallel linear => sum-reduce unshard).

Device algorithm (per core, per batch n):
  1. qT = (WQp/8).T @ Q.T   [128, 2048]   (scale 1/sqrt(dk) folded into WQp)
     kT = WKp.T @ K.T       [128, 2048]
     v  = V @ WVp           [128part(l-tile), 16, 2, dk+1] with a ones
          column appended per head (softmax denominator for free)
  2. scores in "KQ" orientation: S^T[l, q] = kT.T(l-tile) @ qT(q-chunk);
     the two heads run CONCURRENTLY on PE row groups 0-63 / 64-127.
  3. E = exp(S^T) on ScalarE (its only job), PSUM -> SBUF bf16.
  4. attnT_aug[dv+1, q] += v_aug.T(l-tile) @ E accumulated over l-tiles in
     PSUM; row dv holds sum(exp) = softmax denominator.
  5. normalize: att = attnT * bcast(1/den) (DVE; 1/den broadcast across
     partitions via a K=1 PE matmul)
  6. O^T-partial: out[q-tile, :] = att[:, q-tile].T @ WOp, written fp16.

Schedule: a warm-up burst of dummy matmuls un-throttles the PE HAM clock
gate (4/8 -> 8/8) during the initial DMA wait; after that every piece of
projection / normalize / out-projection work is dripped into the
exp-paced attention l-loops so the PE stream never idles (no HAM
re-throttle) and ScalarE runs an uninterrupted exp stream.  All
PSUM->SBUF casts are on DVE, all DMA issue on the SP/GpSimd queues.
"""

import math
import numpy as np
from contextlib import ExitStack
from collections import deque

import concourse.bass as bass
import concourse.tile as tile
from concourse import bacc, mybir
from concourse.bass_utils import run_bass_kernel_spmd

N_CORES = 8
NB, T, D = 2, 2048, 1024
HEADS, DK = 16, 64
HP = 2 * DK          # per-core head-pair width = 128
QC = 512             # query-chunk (matmul moving free dim)
NQC = T // QC        # 4
LTS = 128            # key/l tile (PE partition dim)
NLT = T // LTS       # 16
CK = 128             # contraction chunk for projections
NCK = D // CK        # 8
VW = DK + 1          # v columns per head incl. ones column

F32 = mybir.dt.float32
BF16 = mybir.dt.bfloat16
FP16 = mybir.dt.float16
EXP = mybir.ActivationFunctionType.Exp


def build_program(mm_dt=BF16, out_dt=FP16):
    """Build + compile the SPMD program (identical on all 8 cores)."""
    nc = bacc.Bacc("TRN2", target_bir_lowering=False, debug=False,
                   num_devices=N_CORES)
    # [NB, NQC, CK, NCK*QC]: per (n, chunk) a contiguous [128, 8KB] block
    # so each input chunk is ONE dma_start with 2KB+ descriptors.
    QT = nc.dram_tensor("QT", [NB, NQC, CK, NCK, QC], mm_dt,
                        kind="ExternalInput").ap()
    KT = nc.dram_tensor("KT", [NB, NQC, CK, NCK, QC], mm_dt,
                        kind="ExternalInput").ap()
    VT = nc.dram_tensor("VT", [NB, NQC, CK, NCK, QC], mm_dt,
                        kind="ExternalInput").ap()
    WQp = nc.dram_tensor("WQp", [CK, NCK, HP], mm_dt,
                         kind="ExternalInput").ap()
    WKp = nc.dram_tensor("WKp", [CK, NCK, HP], mm_dt,
                         kind="ExternalInput").ap()
    WVp = nc.dram_tensor("WVp", [CK, NCK, HP], mm_dt,
                         kind="ExternalInput").ap()
    WOp = nc.dram_tensor("WOp", [HP, D], mm_dt, kind="ExternalInput").ap()
    O = nc.dram_tensor("O", [NB, T, D], out_dt, kind="ExternalOutput").ap()

    with tile.TileContext(nc) as tc, ExitStack() as ctx:
        wpool = ctx.enter_context(tc.tile_pool(name="w", bufs=1))
        seq = ctx.enter_context(tc.tile_pool(name="seq", bufs=2))
        inp = ctx.enter_context(tc.tile_pool(name="inp", bufs=11))
        epool = ctx.enter_context(tc.tile_pool(name="e", bufs=8))
        apool = ctx.enter_context(tc.tile_pool(name="att", bufs=3))
        opool = ctx.enter_context(tc.tile_pool(name="o", bufs=6))
        ppool = ctx.enter_context(tc.tile_pool(name="pp", bufs=2, space="PSUM"))
        spool = ctx.enter_context(tc.tile_pool(name="ps", bufs=2, space="PSUM"))
        atpool = ctx.enter_context(tc.tile_pool(name="pa", bufs=1, space="PSUM"))

        # --- static SBUF: weights + constants ---
        wq_s = wpool.tile([CK, NCK, HP], mm_dt)
        wk_s = wpool.tile([CK, NCK, HP], mm_dt)
        wv_s = wpool.tile([CK, NCK, HP], mm_dt)
        wo_s = wpool.tile([HP, D], mm_dt)

        ones_col = wpool.tile([1, DK], F32, name="ones_col")
        nc.vector.memset(ones_col, 1.0)
        ones_col_r = wpool.tile([1, DK], mm_dt, name="ones_col_r")
        nc.vector.tensor_copy(ones_col_r, ones_col)
        # warm-up operand (zeros are fine for dummy matmuls)
        wsrc = wpool.tile([CK, QC], mm_dt, name="wsrc")
        nc.vector.memset(wsrc, 0.125)

        # --- HAM warm-up: dummy matmuls that run during the initial DMA
        # wait. ~8 N=512 matmuls at the cold 1.2GHz clock cover the 3.4us
        # busy window that flips the PE clock gate to 8/8; a few more keep
        # it busy until the first real projection's inputs land. ---
        for wi in range(10):
            pw = ppool.tile([HP, QC], F32, tag="pp", name="warm")
            nc.tensor.matmul(pw, lhsT=wsrc[:, 0:HP], rhs=wsrc,
                             start=True, stop=True)

        # --- staged input chunks (one tile + one DMA per (src, n, c)) ---
        stage = {}

        def dma_issue8(src, n, c, key):
            """Startup chunks: 8 per-ck sub-DMAs split across BOTH queues
            (each issue-stream only sustains ~45GB/s, so landing latency
            scales with stream count), per-ck tiles so each projection
            matmul starts as soon as its own 128KB block lands."""
            for ck in range(NCK):
                cin = inp.tile([CK, QC], mm_dt, tag="cin", bufs=24,
                               name="cin")
                q = nc.sync if ck < 4 else nc.gpsimd
                q.dma_start(out=cin, in_=src[n, c, :, ck, :])
                stage[(key, ck)] = cin

        def dma_in(src, n, c, key, eng=None):
            """Prefetch chunks: 2 half-chunk sub-DMAs (issue-rate cheap:
            the ~650ns DGE issue cost per dma_start is the scarce resource
            mid-run, not bandwidth)."""
            def f():
                q = eng or nc.sync
                for h in range(2):
                    cin = inp.tile([CK, 4, QC], mm_dt, tag="cin4", bufs=18,
                                   name="cin4")
                    q.dma_start(out=cin, in_=src[n, c, :, 4 * h:4 * h + 4, :])
                    for ck in range(4):
                        stage[(key, 4 * h + ck)] = cin[:, ck, :]
            return f

        def qk_mm(w_s, dst, qc, key):
            """Fused projection: 8 accumulating matmuls + one DVE cast."""
            def f():
                ps = ppool.tile([HP, QC], F32, tag="pp", name="ps_proj")
                for ck in range(NCK):
                    nc.tensor.matmul(ps, lhsT=w_s[:, ck, :],
                                     rhs=stage.pop((key, ck)),
                                     start=(ck == 0), stop=(ck == NCK - 1))
                nc.vector.tensor_copy(dst[:, qc * QC:(qc + 1) * QC], ps)
            return [f]

        def v_mm(v_sb, n, c, key):
            """v natural-layout projection for token-chunk c (4 l-tiles);
            one [128, 2, 64] DVE copy per l-tile (ones cols untouched)."""
            cins = {}
            def grab():
                for ck in range(NCK):
                    cins[ck] = stage.pop((key, ck))
            def t(j):
                def f():
                    lt = c * (QC // LTS) + j
                    pv = ppool.tile([LTS, 2, DK], F32, tag="pp", name="pv")
                    for ck in range(NCK):
                        nc.tensor.matmul(
                            pv, lhsT=cins[ck][:, j * LTS:(j + 1) * LTS],
                            rhs=wv_s[:, ck, :],
                            start=(ck == 0), stop=(ck == NCK - 1))
                    nc.vector.tensor_copy(v_sb[:, lt, :, 0:DK], pv)
                return f
            return [grab] + [t(j) for j in range(QC // LTS)]

        def scores_pair(qT_sb, kT_sb, qc, lt):
            """S^T for both heads of (q-chunk, l-tile) into one 2-bank PSUM
            tile; heads run concurrently on PE row groups; single wide exp."""
            ss = spool.tile([LTS, 2 * QC], F32, tag="ss", name="ss")
            for h in range(2):
                nc.tensor.matmul(
                    ss[:, h * QC:(h + 1) * QC],
                    lhsT=kT_sb[DK * h:DK * (h + 1), lt * LTS:(lt + 1) * LTS],
                    rhs=qT_sb[DK * h:DK * (h + 1), qc * QC:(qc + 1) * QC],
                    start=True, stop=True)
            e = epool.tile([LTS, 2 * QC], mm_dt, tag="e", name="e")
            nc.scalar.activation(e, ss, EXP)
            return e

        def av_pair(v_sb, ps_att, e, lt, start, stop):
            for h in range(2):
                nc.tensor.matmul(ps_att[h],
                                 lhsT=v_sb[:, lt, h, :],
                                 rhs=e[:, h * QC:(h + 1) * QC],
                                 start=start, stop=stop)

        def attention_chunk(qT_sb, kT_sb, v_sb, qc, work, slots=None):
            """Emit attention for one q-chunk; `slots` maps l-tile index ->
            thunks that MUST be emitted at that point (production
            deadlines); `work` thunks are consumed evenly across the
            l-tile loop."""
            slots = slots or {}
            ps_att = [atpool.tile([VW, QC], F32, tag=f"pa{h}",
                                  name=f"ps_att{h}") for h in range(2)]
            prev = None
            for lt in range(NLT):
                e = scores_pair(qT_sb, kT_sb, qc, lt)
                for t in slots.get(lt, ()):
                    t()
                if work:
                    take = -(-len(work) // (NLT - lt))
                    for _ in range(min(take, len(work))):
                        work.popleft()()
                if prev is not None:
                    av_pair(v_sb, ps_att, prev, lt - 1, start=(lt == 1),
                            stop=False)
                prev = e
            av_pair(v_sb, ps_att, prev, NLT - 1, start=(NLT == 1), stop=True)
            while work:
                work.popleft()()
            return ps_att

        def norm_thunks(ps_att):
            """Softmax normalization for a finished accumulator pair. The
            PSUM quick-release copies (tA) are emitted INLINE so the next
            chunk's first AV matmul (which reuses the single-buffered
            accumulator) never convoys behind later DVE work. Returns
            (att, [C, D], tDj): C builds the 1/den broadcast, D applies."""
            att_raw = apool.tile([HP, QC], F32, tag="att_raw", name="att_raw")
            att = apool.tile([HP, QC], mm_dt, tag="attT", name="att")
            state = {}

            def tA(h):
                def f():
                    nc.vector.tensor_copy(att_raw[DK * h:DK * (h + 1), :],
                                          ps_att[h][0:DK, :])
                    den_f = apool.tile([1, QC], F32, tag=f"den{h}",
                                       name="den_f")
                    nc.vector.tensor_copy(den_f, ps_att[h][DK:VW, :])
                    state["den%d" % h] = den_f
                return f

            def tC():
                for h in range(2):
                    den_rf = apool.tile([1, QC], F32, tag=f"denr{h}",
                                        name="den_rf")
                    nc.vector.reciprocal_approx_fast(den_rf,
                                                     state["den%d" % h])
                    den_rr = apool.tile([1, QC], mm_dt, tag=f"denrr{h}",
                                        name="den_rr")
                    nc.vector.tensor_copy(den_rr, den_rf)
                    bcp = ppool.tile([DK, QC], F32, tag="pp", name="bc_ps")
                    nc.tensor.matmul(bcp, lhsT=ones_col_r, rhs=den_rr,
                                     start=True, stop=True)
                    state["bc%d" % h] = bcp

            def tD():
                for h in range(2):
                    nc.vector.tensor_mul(att[DK * h:DK * (h + 1), :],
                                         att_raw[DK * h:DK * (h + 1), :],
                                         state["bc%d" % h])

            def tDj(j):
                sl = slice(j * LTS, (j + 1) * LTS)
                for h in range(2):
                    nc.vector.tensor_mul(att[DK * h:DK * (h + 1), sl],
                                         att_raw[DK * h:DK * (h + 1), sl],
                                         state["bc%d" % h][:, sl])

            tA(0)()
            tA(1)()
            return att, [tC, tD], tDj

        def out_proj_thunks(n, att, qc, queues=None):
            """out-projection chunk: 8 (MM + fp16-cast) thunks, DMA per
            q-tile issued from `queues` (round-robin)."""
            queues = queues or [nc.gpsimd]
            box = {}
            thunks = []
            for j in range(QC // LTS):
                for half in range(2):
                    def t(j=j, half=half):
                        qt = qc * (QC // LTS) + j
                        if half == 0:
                            box[j] = opool.tile([LTS, D], out_dt, tag="osb",
                                                name="o_sb")
                        o_sb = box[j]
                        po = ppool.tile([LTS, QC], F32, tag="pp", name="po")
                        nc.tensor.matmul(
                            po, lhsT=att[:, j * LTS:(j + 1) * LTS],
                            rhs=wo_s[:, half * QC:(half + 1) * QC],
                            start=True, stop=True)
                        nc.vector.tensor_copy(
                            o_sb[:, half * QC:(half + 1) * QC], po)
                        if half == 1:
                            queues[j % len(queues)].dma_start(
                                out=O[n, qt * LTS:(qt + 1) * LTS, :],
                                in_=box.pop(j))
                    thunks.append(t)
            return thunks

        def body():
            seqs = []
            for n in range(NB):
                qT_sb = seq.tile([HP, T], mm_dt, tag="qT", name="qT_sb")
                kT_sb = seq.tile([HP, T], mm_dt, tag="kT", name="kT_sb")
                v_sb = seq.tile([LTS, NLT, 2, VW], mm_dt, tag="v",
                                name="v_sb")
                nc.vector.memset(v_sb[:, :, :, DK], 1.0)
                seqs.append((qT_sb, kT_sb, v_sb))

            def kv_proj(n, c):
                """3 drip thunks: k-proj burst, then v j-tiles 0-1, 2-3."""
                k = qk_mm(wk_s, seqs[n][1], c, ("k", n, c))
                v = v_mm(seqs[n][2], n, c, ("v", n, c))
                return [k[0], lambda: [t() for t in v[0:3]],
                        lambda: [t() for t in v[3:5]]]

            def dmas(*specs):
                """One thunk per chunk DMA (8 issues each) on gpsimd."""
                return [dma_in(src, n, c, (pfx, n, c), eng=nc.gpsimd)
                        for (src, n, c, pfx) in specs]

            # --- startup: weights + the three chunk-0 inputs split across
            # BOTH queues (parallel pull, projections start per-ck as
            # blocks land); chunk-1/2 K/V prefetched with cheap half-chunk
            # DMAs. Warm-up matmuls above cover the DMA wait. ---
            nc.sync.dma_start(out=wq_s, in_=WQp)
            nc.gpsimd.dma_start(out=wv_s, in_=WVp)
            nc.sync.dma_start(out=wk_s, in_=WKp)
            nc.gpsimd.dma_start(out=wo_s, in_=WOp)
            dma_issue8(QT, 0, 0, ("q", 0, 0))
            dma_issue8(KT, 0, 0, ("k", 0, 0))
            dma_issue8(VT, 0, 0, ("v", 0, 0))
            dma_in(KT, 0, 1, ("k", 0, 1), eng=nc.gpsimd)()
            dma_in(VT, 0, 1, ("v", 0, 1), eng=nc.gpsimd)()
            dma_in(KT, 0, 2, ("k", 0, 2))()
            dma_in(VT, 0, 2, ("v", 0, 2))()

            for t in qk_mm(wq_s, seqs[0][0], 0, ("q", 0, 0)):
                t()
            for t in qk_mm(wk_s, seqs[0][1], 0, ("k", 0, 0)):
                t()
            for t in v_mm(seqs[0][2], 0, 0, ("v", 0, 0)):
                t()

            # per-chunk deadline slots and drip work
            kv01 = kv_proj(0, 1)
            kv02 = kv_proj(0, 2)
            kv03 = kv_proj(0, 3)
            slots_ci0 = {
                0: dmas((KT, 0, 3, "k"), (VT, 0, 3, "v")),
                2: [kv01[0]],
                4: [kv01[1]],
                5: [kv01[2]],
                3: dmas((QT, 0, 1, "q")),
                6: [kv02[0]],
                7: [kv02[1]],
                8: [kv02[2]],
                9: dmas((KT, 1, 0, "k"), (VT, 1, 0, "v")),
                10: [kv03[0]],
                11: [kv03[1]],
                12: [kv03[2]],
                13: qk_mm(wq_s, seqs[0][0], 1, ("q", 0, 1)),
                14: dmas((QT, 0, 2, "q"), (KT, 1, 1, "k"), (VT, 1, 1, "v")),
            }
            dma_extra = {
                1: dmas((KT, 1, 2, "k"), (VT, 1, 2, "v"), (QT, 0, 3, "q")),
                2: dmas((KT, 1, 3, "k"), (VT, 1, 3, "v"), (QT, 1, 0, "q")),
                3: dmas((QT, 1, 1, "q")),
                4: dmas((QT, 1, 2, "q")),
                5: dmas((QT, 1, 3, "q")),
            }
            # future projection work dripped per chunk index ci=1..7
            proj_extra = {
                1: qk_mm(wq_s, seqs[0][0], 2, ("q", 0, 2)) + kv_proj(1, 0),
                2: qk_mm(wq_s, seqs[0][0], 3, ("q", 0, 3)) + kv_proj(1, 1),
                3: qk_mm(wq_s, seqs[1][0], 0, ("q", 1, 0)) + kv_proj(1, 2),
                4: qk_mm(wq_s, seqs[1][0], 1, ("q", 1, 1)),
                5: qk_mm(wq_s, seqs[1][0], 2, ("q", 1, 2)),
                6: qk_mm(wq_s, seqs[1][0], 3, ("q", 1, 3)),
            }
            # batch-1 chunk-3 K/V projection has a hard deadline (scores of
            # chunk (1,0) l-tiles 12-15): pin it in slots of ci=4.
            kv13 = kv_proj(1, 3)
            slots_ci4 = {0: [kv13[0]], 1: [kv13[1]], 2: [kv13[2]]}

            pend_norm = None
            pend_out = None
            for ci in range(NB * NQC):
                n, qc = divmod(ci, NQC)
                qT_sb, kT_sb, v_sb = seqs[n]
                work = deque()
                if pend_norm:
                    work.extend(pend_norm)          # bc + mul
                work.extend(dma_extra.get(ci, ()))
                if pend_out is not None:
                    work.extend(out_proj_thunks(pend_out[2], pend_out[0],
                                                pend_out[1]))
                work.extend(proj_extra.get(ci, ()))
                slots = slots_ci0 if ci == 0 else \
                    (slots_ci4 if ci == 4 else None)
                ps_att = attention_chunk(qT_sb, kT_sb, v_sb, qc, work, slots)
                att, pend_norm, pend_tdj = norm_thunks(ps_att)
                pend_out = (att, qc, n)
            # pipelined tail: per q-tile, normalize the slice then
            # immediately out-project it; final DMAs alternate queues.
            # Dummy matmuls keep the PE HAM clock gate at 8/8 through the
            # norm-chain waits so the out-projection runs at 2.4GHz.
            def warm(k):
                for _ in range(k):
                    pw = ppool.tile([HP, QC], F32, tag="pp", name="warm")
                    nc.tensor.matmul(pw, lhsT=wsrc[:, 0:HP], rhs=wsrc,
                                     start=True, stop=True)
            warm(4)                     # PE busy while DVE runs tA + recip
            pend_norm[0]()              # tC (bc matmuls)
            pend_norm[1]()              # tD
            warm(6)                     # PE busy while DVE runs tD
            op = out_proj_thunks(pend_out[2], pend_out[0], pend_out[1],
                                 queues=[nc.gpsimd, nc.sync])
            for j in range(QC // LTS):
                op[2 * j]()
                op[2 * j + 1]()
                warm(2)

        body()

    nc.compile()
    return nc


_CACHED = {}


def _get_program(key=("bf16",)):
    if key not in _CACHED:
        _CACHED[key] = build_program()
    return _CACHED[key]


def prep_inputs(Q, K, V, WQ, WK, WV, WO, bO):
    """Host-side shard prep: transposes + per-core weight slices."""
    import ml_dtypes
    wire = ml_dtypes.bfloat16
    Q = np.asarray(Q, dtype=np.float32)
    K = np.asarray(K, dtype=np.float32)
    V = np.asarray(V, dtype=np.float32)
    WQ = np.asarray(WQ, dtype=np.float32)
    WK = np.asarray(WK, dtype=np.float32)
    WV = np.asarray(WV, dtype=np.float32)
    WO = np.asarray(WO, dtype=np.float32)

    def blockT(X):
        # [N, T, D] -> X^T blocked [NB, NQC, CK, NCK, QC]; per (n, qc) the
        # [CK, NCK*QC] block is contiguous (one DMA, 8KB per partition)
        Xt = np.swapaxes(X, 1, 2).reshape(NB, NCK, CK, NQC, QC)
        return np.ascontiguousarray(
            Xt.transpose(0, 3, 2, 1, 4)).astype(wire)

    QT = blockT(Q)
    KT = blockT(K)
    VT = blockT(V)
    scale = 1.0 / math.sqrt(DK)

    def wblk(w):
        # [D, HP] -> [CK, NCK, HP] (d = k*CK + c -> [c, k, m]), contiguous
        return np.ascontiguousarray(
            w.reshape(NCK, CK, HP).transpose(1, 0, 2)).astype(wire)

    in_maps = []
    for p in range(N_CORES):
        sl = slice(HP * p, HP * (p + 1))
        in_maps.append({
            "QT": QT, "KT": KT, "VT": VT,
            "WQp": wblk(np.ascontiguousarray(WQ[:, sl]) * scale),
            "WKp": wblk(np.ascontiguousarray(WK[:, sl])),
            "WVp": wblk(np.ascontiguousarray(WV[:, sl])),
            "WOp": np.ascontiguousarray(WO[sl, :]).astype(wire),
        })
    return in_maps


def kernel(Q, K, V, WQ, WK, WV, WO, bO):
    nc = _get_program()
    in_maps = prep_inputs(Q, K, V, WQ, WK, WV, WO, bO)
    res = run_bass_kernel_spmd(nc, in_maps, list(range(N_CORES)))
    acc = np.zeros((NB, T, D), np.float32)
    for p in range(N_CORES):
        acc += res.results[p]["O"].astype(np.float32)
    return acc + np.asarray(bO, dtype=np.float32)
